# revision 2
# baseline (speedup 1.0000x reference)
"""Trainium2 Bass kernel for nn_DifferentiableFeatureExtractor (v2).

Strategy vs baseline: exact (untruncated) EMAs as single chunk scans with
PE-matmul partition carries + geometric-tail fixes; all scale factors
folded symbolically and applied on the ACT engine at combine/store points;
plain add/sub/mult tensor work offloaded to GpSimd; DVE keeps scans,
compares, reciprocals, PSUM-reading ops and the max/min window chains.
Constant tiles (shift matrices, fp32 iotas, geometric decay tables) are
precomputed on the host and shipped as one DRAM input.
"""
import math

import numpy as np

import concourse.bacc as bacc
from concourse.bass_types import AP as BassAP
import concourse.mybir as mybir
from concourse import tile as ctile
from concourse.bass_utils import run_bass_kernel_spmd

F32 = mybir.dt.float32
Alu = mybir.AluOpType
Act = mybir.ActivationFunctionType

T = 1048576
NCORES = 8
S = T // NCORES            # 131072
P = 128
CH = 1040                  # chunk cols per partition
HP = 256                   # per-partition AP halo cols (input loads)
W = HP + CH                # 1296
EXT = P * CH               # 133120
HALO = EXT - S             # 2048
DLEN = HP + EXT            # 133376
C0 = HP                    # chunk start col
NROWS = 30

GTHRESH = 16.0             # geometric fix window: c^w <= e^-16


def g_width(a):
    c = 1.0 - a
    return min(CH, int(math.ceil(GTHRESH / (-math.log(c)))))


class KB:
    def __init__(self, alphas, anchor):
        self.alphas = [float(a) for a in alphas]
        self.anchor = float(anchor)
        nc = bacc.Bacc(None, target_bir_lowering=False)
        self.nc = nc
        self.DC = nc.dram_tensor("DC", [DLEN], F32, kind="ExternalInput")
        self.DH = nc.dram_tensor("DH", [DLEN], F32, kind="ExternalInput")
        self.DL = nc.dram_tensor("DL", [DLEN], F32, kind="ExternalInput")
        # CONST layout (cols), ordered by first use so the initial DMA
        # fragment (cvals+Sh1+G_A18+G_A80) unblocks the first scans fast:
        # CVALS[0:21] Sh1[21:149] G tables, Sh1e, Ident, M2, TG.
        self.off_cvals = 0     # 20 c-values + ones col = 21 cols
        self.off_sh1 = 21
        goff = self.off_sh1 + 128
        self.g_off = {}
        self.g_w = {}
        seen = {}
        for i in [0, 3] + list(range(20)):
            a = self.alphas[i]
            c = round(1.0 - a, 12)
            if c not in seen:
                w = g_width(a)
                seen[c] = (goff, w)
                goff += w
            self.g_off[i], self.g_w[i] = seen[c]
        self.dma1_cols = self.g_off[3] + self.g_w[3]   # through G_A80
        self.off_sh1e = goff
        self.off_ident = goff + 128
        self.off_m2 = goff + 256
        self.off_tg = goff + 384
        self.nconst = self.off_tg + CH
        self.CONST = nc.dram_tensor(
            "CONST", [P * self.nconst], F32, kind="ExternalInput"
        )
        self.OUT = nc.dram_tensor("OUT", [NROWS * EXT], F32, kind="ExternalOutput")
        self.DIAG = nc.dram_tensor("DIAG", [2], F32, kind="ExternalOutput")
        self.free_big = []
        self.n_big = 0
        self.free_row = []
        self.n_row = 0
        self.free_row129 = []
        self.n_row129 = 0
        self.free_small = []
        self.n_small = 0

    # ---- host-side constant block ----
    def host_const(self):
        a = np.zeros((P, self.nconst), np.float32)
        ii = np.arange(P)
        for i, al in enumerate(self.alphas):
            a[:, self.off_cvals + i] = np.float32(1.0 - al)
        a[:, self.off_cvals + 20] = 1.0
        # Sh1: matmul(out, lhsT, rhs): out[po,f] = sum_pi lhsT[pi,po] rhs[pi,f]
        sh1 = np.zeros((P, P), np.float32)
        sh1[ii[:-1], ii[1:]] = 1.0          # out[po] = rhs[po-1]
        a[:, self.off_sh1 : self.off_sh1 + P] = sh1
        done = set()
        for i, al in enumerate(self.alphas):
            off, w = self.g_off[i], self.g_w[i]
            if off in done:
                continue
            done.add(off)
            c = 1.0 - al
            a[:, off : off + w] = np.power(
                c, np.arange(1, w + 1, dtype=np.float64)
            ).astype(np.float32)[None, :]
        sh1e = sh1.copy()
        sh1e[0, 0] = 1.0                    # partition 0 reads its own tail
        a[:, self.off_sh1e : self.off_sh1e + P] = sh1e
        a[:, self.off_ident : self.off_ident + P] = np.eye(P, dtype=np.float32)
        cF = (1.0 - self.alphas[6]) ** CH
        sh2 = np.zeros((P, P), np.float32)
        sh2[ii[:-2], ii[2:]] = 1.0
        a[:, self.off_m2 : self.off_m2 + P] = sh1 + np.float32(cF) * sh2
        a[:, self.off_tg : self.off_tg + CH] = (
            ii[:, None] * CH + np.arange(CH)[None, :]
        ).astype(np.float32)
        return np.ascontiguousarray(a.reshape(-1))

    # ---- tile management ----
    def big(self):
        if self.free_big:
            return self.free_big.pop(0)
        t = self.pool.tile([P, W], F32, tag=f"big{self.n_big}")
        self.n_big += 1
        return t

    def rel(self, *ts):
        for t in ts:
            self.free_big.append(t)

    def row(self):
        if self.free_row:
            return self.free_row.pop()
        t = self.spool.tile([1, P], F32, tag=f"row{self.n_row}")
        self.n_row += 1
        return t

    def relr(self, *ts):
        for t in ts:
            self.free_row.append(t)

    def row129(self):
        if self.free_row129:
            return self.free_row129.pop()
        t = self.spool.tile([1, P + 1], F32, tag=f"row129_{self.n_row129}")
        self.n_row129 += 1
        return t

    def relr129(self, *ts):
        for t in ts:
            self.free_row129.append(t)

    def small(self):
        if self.free_small:
            return self.free_small.pop()
        t = self.spool.tile([P, 1], F32, tag=f"small{self.n_small}")
        self.n_small += 1
        return t

    def rels(self, *ts):
        for t in ts:
            self.free_small.append(t)

    # ---- IO ----
    def load_series(self, dram, eng="sync"):
        nc = self.nc
        t = self.big()
        base = dram[0:DLEN].rearrange("(a b) -> a b", a=1, b=DLEN)
        src_ap = BassAP(base.tensor, 0, [[CH, P], [1, W]])
        getattr(nc, eng).dma_start(out=t[:, 0:W], in_=src_ap)
        return t

    def store_row(self, r, t):
        nc = self.nc
        nc.sync.dma_start(
            out=self.OUT[r * EXT : (r + 1) * EXT].rearrange(
                "(p w) -> p w", p=P, w=CH
            ),
            in_=t[:, C0:W],
        )

    # ---- building blocks ----
    def mm(self, out, lhsT, rhs):
        self.nc.tensor.matmul(out, lhsT, rhs, start=True, stop=True)

    def scan_u(self, xt, i, lo=C0):
        """unscaled EMA scan over chunk cols (initial 0); no carry yet."""
        nc = self.nc
        s = self.big()
        cbc = self.CT[:, self.off_cvals + i : self.off_cvals + i + 1].broadcast_to(
            [P, W - lo]
        )
        nc.vector.tensor_tensor_scan(
            out=s[:, lo:W], data0=cbc, data1=xt[:, lo:W],
            initial=0.0, op0=Alu.mult, op1=Alu.add,
        )
        return s

    def fix_u(self, s, i):
        """partition carry: s += S_in * c^(t-C0+1) on the geometric window."""
        nc = self.nc
        mmat = self.M2 if i == 6 else self.Sh1
        pcar = self.pscol.tile([P, 1], F32, tag="pscol")
        self.mm(pcar[:, 0:1], mmat, s[:, W - 1 : W])
        w = self.g_w[i]
        off = self.g_off[i]
        nc.vector.scalar_tensor_tensor(
            out=s[:, C0 : C0 + w], in0=self.CT[:, off : off + w],
            scalar=pcar[:, 0:1], in1=s[:, C0 : C0 + w],
            op0=Alu.mult, op1=Alu.add,
        )
        return s

    def ema_u(self, xt, i):
        return self.fix_u(self.scan_u(xt, i), i)

    def extend_left(self, t, hw):
        """fill t[:, C0-hw:C0] with prev partition's last hw chunk cols
        (partition 0 gets its own tail values: finite, decays in halo)."""
        nc = self.nc
        ph = self.pshalo.tile([P, hw], F32, tag="psh")
        self.mm(ph[:, 0:hw], self.Sh1e, t[:, W - hw : W])
        nc.scalar.copy(t[:, C0 - hw : C0], ph[:, 0:hw])

    def act_mul(self, src, scale, lo=C0, keep=False):
        nc = self.nc
        out = self.big()
        nc.scalar.mul(out[:, lo:W], src[:, lo:W], float(scale))
        return out

    def act_affine(self, src, scale, bias_ap, lo=C0):
        nc = self.nc
        out = self.big()
        nc.scalar.activation(
            out[:, lo:W], src[:, lo:W], Act.Identity,
            bias=bias_ap, scale=float(scale),
        )
        return out

    def gp_tt(self, a, b, op, lo=C0, sa=None, sb=None):
        nc = self.nc
        out = self.big()
        ia = a[:, lo:W] if sa is None else a[:, lo - sa : W - sa]
        ib = b[:, lo:W] if sb is None else b[:, lo - sb : W - sb]
        nc.gpsimd.tensor_tensor(out=out[:, lo:W], in0=ia, in1=ib, op=op)
        return out

    def dve_tt(self, a, b, op, lo=C0, sa=None, sb=None):
        nc = self.nc
        out = self.big()
        ia = a[:, lo:W] if sa is None else a[:, lo - sa : W - sa]
        ib = b[:, lo:W] if sb is None else b[:, lo - sb : W - sb]
        nc.vector.tensor_tensor(out=out[:, lo:W], in0=ia, in1=ib, op=op)
        return out

    # ---- full pipeline ----
    def build(self):
        nc = self.nc
        with ctile.TileContext(nc) as tc:
            with tc.tile_pool(name="big", bufs=1) as pool, tc.tile_pool(
                name="small", bufs=1
            ) as spool, tc.tile_pool(
                name="psc", bufs=2, space="PSUM"
            ) as pscol, tc.tile_pool(
                name="psh", bufs=4, space="PSUM"
            ) as pshalo, tc.tile_pool(name="psr", bufs=2, space="PSUM") as psrow:
                self.pool = pool
                self.spool = spool
                self.pscol = pscol
                self.psrow = psrow
                self.pshalo = pshalo
                self.emit()
        nc.finalize()
        return nc

    def chain_step(self, cur, sh, lo, op):
        dst = self.big()
        self.nc.vector.tensor_tensor(
            out=dst[:, lo:W], in0=cur[:, lo:W], in1=cur[:, lo - sh : W - sh],
            op=op,
        )
        return dst

    def stt(self, in0, scal, in1, op0=Alu.mult, op1=Alu.add, lo=C0, s0=None):
        """DVE fused (in0*scal) op1 in1 -> new tile."""
        out = self.big()
        i0 = in0[:, lo:W] if s0 is None else in0[:, lo - s0 : W - s0]
        self.nc.vector.scalar_tensor_tensor(
            out=out[:, lo:W], in0=i0, scalar=scal, in1=in1[:, lo:W],
            op0=op0, op1=op1,
        )
        return out

    def kdj_store(self, sk, sd, inner, ik, idd, rows):
        ak, ad = self.alphas[ik], self.alphas[idd]
        if rows[0] is not None:
            Kt = self.act_mul(sk, 100.0 * ak)
            self.store_row(rows[0], Kt)
            self.rel(Kt)
            Dt = self.act_mul(sd, 100.0 * ak * ad)
            self.store_row(rows[1], Dt)
            self.rel(Dt)
        Jt = self.act_mul(inner, 300.0 * ak)
        self.store_row(rows[2], Jt)
        self.rel(Jt, sk, sd)

    def emit(self):
        nc = self.nc
        AL = self.alphas

        # ---- constant block: tiny fragment first, bulk later ----
        self.CT = self.spool.tile([P, self.nconst], F32, tag="ct")
        nct = self.nconst
        base = self.CONST[0 : P * nct].rearrange("(p n) -> p n", p=P, n=nct)
        d1c = self.dma1_cols
        src1 = BassAP(base.tensor, 0, [[nct, P], [1, d1c]])
        nc.sync.dma_start(out=self.CT[:, 0:d1c], in_=src1)
        Ct = self.load_series(self.DC)
        Ht = self.load_series(self.DH)
        Lt = self.load_series(self.DL)
        src2 = BassAP(base.tensor, d1c, [[nct, P], [1, nct - d1c]])
        nc.sync.dma_start(out=self.CT[:, d1c:nct], in_=src2)
        o = self.off_sh1
        self.Sh1 = self.CT[:, o : o + 128]
        self.Sh1e = self.CT[:, self.off_sh1e : self.off_sh1e + 128]
        self.Ident = self.CT[:, self.off_ident : self.off_ident + 128]
        self.M2 = self.CT[:, self.off_m2 : self.off_m2 + 128]
        self.TG = self.CT[:, self.off_tg : self.off_tg + CH]
        self.ones11 = self.CT[0:1, self.off_cvals + 20 : self.off_cvals + 21]
        self.nanch = self.spool.tile([P, 1], F32, tag="c_nanch")
        nc.gpsimd.memset(self.nanch[:, :], -self.anchor)
        self.m50 = self.spool.tile([P, 1], F32, tag="c_m50")
        nc.gpsimd.memset(self.m50[:, :], -50.0)

        # ============ head: chains (critical 204-lane) + C scans ========
        h = {}
        l = {}
        h[2] = self.chain_step(Ht, 1, C0 - 202, Alu.max)
        l[2] = self.chain_step(Lt, 1, C0 - 202, Alu.min)
        s1 = self.scan_u(Ct, 0)
        h[4] = self.chain_step(h[2], 2, C0 - 200, Alu.max)
        l[4] = self.chain_step(l[2], 2, C0 - 200, Alu.min)
        self.rel(h[2], l[2])
        se1 = self.scan_u(Ct, 3)
        dev = self.big()
        nc.scalar.activation(
            dev[:, C0:W], Ct[:, C0:W], Act.Identity, bias=self.nanch[:, 0:1]
        )
        dev2 = self.big()
        nc.scalar.activation(
            dev2[:, C0:W], Ct[:, C0:W], Act.Square, bias=self.nanch[:, 0:1]
        )
        h[8] = self.chain_step(h[4], 4, C0 - 196, Alu.max)
        l[8] = self.chain_step(l[4], 4, C0 - 196, Alu.min)
        self.rel(h[4], l[4])
        h[9] = self.chain_step(h[8], 1, C0 - 195, Alu.max)
        l[9] = self.chain_step(l[8], 1, C0 - 195, Alu.min)
        self.rel(h[8], l[8])
        self.fix_u(s1, 0)
        self.fix_u(se1, 3)
        den9 = self.gp_tt(h[9], l[9], Alu.subtract)
        num9 = self.gp_tt(Ct, l[9], Alu.subtract)
        h[18] = self.chain_step(h[9], 9, C0 - 186, Alu.max)
        l[18] = self.chain_step(l[9], 9, C0 - 186, Alu.min)
        self.rel(h[9], l[9])
        s2 = self.scan_u(s1, 1)
        se2 = self.scan_u(se1, 4)
        den18 = self.gp_tt(h[18], l[18], Alu.subtract)
        num18 = self.gp_tt(Ct, l[18], Alu.subtract)
        h[36] = self.chain_step(h[18], 18, C0 - 168, Alu.max)
        l[36] = self.chain_step(l[18], 18, C0 - 168, Alu.min)
        self.rel(h[18], l[18])
        rcp9 = self.big()
        nc.vector.reciprocal_approx_fast(out=rcp9[:, C0:W], in_=den9[:, C0:W])
        self.rel(den9)
        h[72] = self.chain_step(h[36], 36, C0 - 132, Alu.max)
        l[72] = self.chain_step(l[36], 36, C0 - 132, Alu.min)
        den36 = self.gp_tt(h[36], l[36], Alu.subtract)
        num36 = self.gp_tt(Ct, l[36], Alu.subtract)
        self.rel(h[36], l[36])
        h[144] = self.chain_step(h[72], 72, C0 - 60, Alu.max)
        l[144] = self.chain_step(l[72], 72, C0 - 60, Alu.min)
        self.rel(h[72], l[72])
        rcp18 = self.big()
        nc.vector.reciprocal_approx_fast(out=rcp18[:, C0:W], in_=den18[:, C0:W])
        self.rel(den18)
        h[204] = self.chain_step(h[144], 60, C0, Alu.max)
        l[204] = self.chain_step(l[144], 60, C0, Alu.min)
        self.rel(h[144], l[144])
        rsv9 = self.gp_tt(num9, rcp9, Alu.mult)
        self.rel(num9, rcp9)
        den204 = self.gp_tt(h[204], l[204], Alu.subtract)
        num204 = self.gp_tt(Ct, l[204], Alu.subtract)
        self.rel(h[204], l[204])
        rcp204 = self.big()
        nc.vector.reciprocal_approx_fast(
            out=rcp204[:, C0:W], in_=den204[:, C0:W]
        )
        self.rel(den204)
        rsv204 = self.gp_tt(num204, rcp204, Alu.mult)
        self.rel(num204, rcp204)

        # ============ KDJ lane 204 (critical) + others interleaved ======
        sk204 = self.scan_u(rsv204, 6)
        self.rel(rsv204)
        self.fix_u(sk204, 6)
        sd204 = self.scan_u(sk204, 7)
        self.fix_u(sd204, 7)
        a6, a7 = AL[6], AL[7]
        i1 = self.stt(sd204, -2.0 * a7 / 3.0, sk204)   # s_k - (2/3) a_d s_d
        self.kdj_store(sk204, sd204, i1, 6, 7, (9, 10, 11))
        sk9 = self.scan_u(rsv9, 10)
        self.rel(rsv9)
        self.fix_u(sk9, 10)
        sd9 = self.scan_u(sk9, 11)
        self.fix_u(sd9, 11)
        i3 = self.stt(sd9, -2.0 * AL[11] / 3.0, sk9)
        self.kdj_store(sk9, sd9, i3, 10, 11, (15, 16, 17))

        rsv18 = self.gp_tt(num18, rcp18, Alu.mult)
        self.rel(num18, rcp18)
        rcp36 = self.big()
        nc.vector.reciprocal_approx_fast(out=rcp36[:, C0:W], in_=den36[:, C0:W])
        self.rel(den36)
        sk18 = self.scan_u(rsv18, 8)
        self.rel(rsv18)
        self.fix_u(sk18, 8)
        sd18 = self.scan_u(sk18, 9)
        self.fix_u(sd18, 9)
        i2 = self.stt(sd18, -2.0 * AL[9] / 3.0, sk18)
        self.kdj_store(sk18, sd18, i2, 8, 9, (12, 13, 14))
        rsv36 = self.gp_tt(num36, rcp36, Alu.mult)
        self.rel(num36, rcp36)

        # ---- C/E2 combines + stdp + ratios (off critical path) ----
        self.fix_u(s2, 1)
        self.fix_u(se2, 4)
        s3 = self.scan_u(s2, 2)
        se3 = self.scan_u(se2, 5)
        self.fix_u(s3, 2)
        self.fix_u(se3, 5)
        onesbc = self.CT[
            :, self.off_cvals + 20 : self.off_cvals + 21
        ].broadcast_to([P, CH])
        cs1 = self.big()
        nc.vector.tensor_tensor_scan(
            out=cs1[:, C0:W], data0=onesbc, data1=dev[:, C0:W],
            initial=0.0, op0=Alu.mult, op1=Alu.add,
        )
        cs2 = self.big()
        nc.vector.tensor_tensor_scan(
            out=cs2[:, C0:W], data0=onesbc, data1=dev2[:, C0:W],
            initial=0.0, op0=Alu.mult, op1=Alu.add,
        )
        self.rel(dev, dev2)
        a0, a3 = AL[0], AL[3]
        pd = self.act_mul(s2, a0)
        pd2 = self.act_mul(se2, a3)
        d = self.gp_tt(s1, pd, Alu.subtract)
        d2 = self.gp_tt(se1, pd2, Alu.subtract)
        self.rel(pd, pd2, s1, se1)
        pu = self.act_mul(s3, a0 * a0 / 3.0)
        pu2 = self.act_mul(se3, a3 * a3 / 3.0)
        u = self.gp_tt(d, pu, Alu.add)
        u2 = self.gp_tt(d2, pu2, Alu.add)
        self.rel(pu, pu2, d, d2, s2, s3, se2, se3)
        TEMA3t = self.act_mul(u, 3.0 * a0)
        self.store_row(4, TEMA3t)
        self.extend_left(u, 8)
        self.extend_left(u2, 8)

        def cumsum_head(cs):
            t18 = self.big()
            nc.vector.tensor_scalar(
                out=t18[:, 0:18], in0=cs[:, W - 18 : W],
                scalar1=cs[:, W - 1 : W], scalar2=None, op0=Alu.subtract,
            )
            ph = self.pshalo.tile([P, 18], F32, tag="psh")
            self.mm(ph[:, 0:18], self.Sh1, t18[:, 0:18])
            self.rel(t18)
            S = self.big()
            nc.vector.tensor_tensor(
                out=S[:, C0 : C0 + 18], in0=cs[:, C0 : C0 + 18],
                in1=ph[:, 0:18], op=Alu.subtract,
            )
            nc.gpsimd.tensor_tensor(
                out=S[:, C0 + 18 : W], in0=cs[:, C0 + 18 : W],
                in1=cs[:, C0 : W - 18], op=Alu.subtract,
            )
            self.rel(cs)
            return S

        S1 = cumsum_head(cs1)
        S2 = cumsum_head(cs2)
        t1 = self.big()
        nc.scalar.activation(
            t1[:, C0:W], S1[:, C0:W], Act.Square, scale=1.0 / math.sqrt(18.0)
        )
        vr = self.gp_tt(S2, t1, Alu.subtract)
        self.rel(S1, S2, t1)
        vrr = self.big()
        nc.scalar.activation(vrr[:, C0:W], vr[:, C0:W], Act.Relu)
        self.rel(vr)
        DIS = self.big()
        nc.scalar.activation(
            DIS[:, C0:W], vrr[:, C0:W], Act.Sqrt, scale=1.0 / 18.0
        )
        self.rel(vrr)
        TEU3 = self.gp_tt(TEMA3t, DIS, Alu.add)
        self.store_row(3, TEU3)
        TED = self.gp_tt(TEMA3t, DIS, Alu.subtract)
        self.store_row(5, TED)
        self.rel(TEU3, TED, DIS, TEMA3t)

        # ratios: T1s gates w (critical-ish) -> DVE recips, Gp muls early
        ab1 = self.big()
        nc.scalar.activation(ab1[:, C0:W], u[:, C0 - 1 : W - 1], Act.Abs)
        ab6 = self.big()
        nc.scalar.activation(ab6[:, C0:W], u[:, C0 - 6 : W - 6], Act.Abs)
        ab26 = self.big()
        nc.scalar.activation(ab26[:, C0:W], u2[:, C0 - 6 : W - 6], Act.Abs)
        rr1 = self.big()
        nc.vector.reciprocal_approx_fast(out=rr1[:, C0:W], in_=ab1[:, C0:W])
        rr6 = self.big()
        nc.vector.reciprocal_approx_fast(out=rr6[:, C0:W], in_=ab6[:, C0:W])
        rr26 = self.big()
        nc.vector.reciprocal_approx_fast(out=rr26[:, C0:W], in_=ab26[:, C0:W])
        self.rel(ab1, ab6, ab26)
        dt1 = self.gp_tt(u, u, Alu.subtract, sb=1)
        dt6 = self.gp_tt(u, u, Alu.subtract, sb=6)
        dt26 = self.gp_tt(u2, u2, Alu.subtract, sb=6)
        self.rel(u, u2)
        T1s = self.gp_tt(dt1, rr1, Alu.mult)
        self.store_row(6, T1s)
        T3s = self.gp_tt(dt6, rr6, Alu.mult)
        self.store_row(8, T3s)
        T2s = self.gp_tt(dt26, rr26, Alu.mult)
        self.store_row(7, T2s)
        self.rel(dt1, dt6, dt26, rr1, rr6, rr26)

        # remaining KDJ lane (win36, JN3 only)
        sk36 = self.scan_u(rsv36, 12)
        self.rel(rsv36)
        self.fix_u(sk36, 12)
        sd36 = self.scan_u(sk36, 13)
        self.fix_u(sd36, 13)
        iN = self.stt(sd36, -2.0 * AL[13] / 3.0, sk36)
        self.kdj_store(sk36, sd36, iN, 12, 13, (None, None, 18))
        self.rel(iN)

        # ============ JX critical lane: w -> scans -> EMAJX -> MA =======
        a8, a10 = AL[8], AL[10]
        t1x = self.dve_tt(i3, T1s, Alu.mult)
        w1 = self.stt(i2, a8 / a6, i1)
        w = self.stt(t1x, a10 / a6, w1)
        self.rel(w1, t1x)
        f1u = self.dve_tt(i2, T3s, Alu.mult)
        f2u = self.dve_tt(i1, T2s, Alu.mult)
        self.rel(i1, i2, i3, T1s, T2s, T3s)
        a14, a15, a16, a17, a18 = AL[14], AL[15], AL[16], AL[17], AL[18]
        sjx = self.scan_u(w, 14)
        self.fix_u(sjx, 14)
        EJXt = self.act_mul(sjx, 300.0 * a6 * a14)
        self.store_row(22, EJXt)
        self.rel(EJXt)
        sf1 = self.scan_u(f1u, 15)
        self.fix_u(sf1, 15)
        EF1t = self.act_mul(sf1, 300.0 * a8 * a15)
        self.store_row(23, EF1t)
        self.rel(EF1t)
        sf2 = self.scan_u(f2u, 16)
        self.fix_u(sf2, 16)
        EF2t = self.act_mul(sf2, 300.0 * a6 * a16)
        self.store_row(24, EF2t)
        self.rel(EF2t)
        # JX needs no scans: compute + extend while scans run
        z1 = self.stt(f1u, 6.0 * a8 / a6, w)
        z2 = self.stt(f2u, 6.0, z1)
        self.rel(z1)
        JX = self.big()
        nc.scalar.activation(
            JX[:, C0:W], z2[:, C0:W], Act.Identity,
            bias=self.m50[:, 0:1], scale=300.0 * a6,
        )
        self.rel(z2)
        self.store_row(27, JX)
        self.extend_left(JX, 2)
        z3 = self.stt(sf1, 6.0 * (a8 * a15) / (a6 * a14), sjx)
        z4 = self.stt(sf2, 6.0 * a16 / a14, z3)
        self.rel(z3)
        EMAJX = self.big()
        nc.scalar.activation(
            EMAJX[:, C0:W], z4[:, C0:W], Act.Identity,
            bias=self.m50[:, 0:1], scale=300.0 * a6 * a14,
        )
        self.rel(z4)
        self.store_row(28, EMAJX)
        self.extend_left(EMAJX, 2)
        self.rel(sjx, sf1, sf2)

        # crosses immediately (gate everything downstream)
        B = {}
        for b, (o1, o2) in (("dn", (Alu.is_ge, Alu.is_lt)),
                            ("up", (Alu.is_le, Alu.is_gt))):
            g = self.dve_tt(JX, EMAJX, o1)
            ll_ = self.dve_tt(JX, EMAJX, o2, sa=1, sb=1)
            m_ = self.dve_tt(g, ll_, Alu.logical_or)
            self.rel(g, ll_)
            B[b] = {"m": m_}
            B[b]["dmask"] = self.gp_tt(Ct, m_, Alu.mult)
        for b in ("dn", "up"):
            m_ = B[b]["m"]
            cnt_s = self.big()
            nc.vector.tensor_tensor_scan(
                out=cnt_s[:, C0:W], data0=m_[:, C0:W], data1=m_[:, C0:W],
                initial=0.0, op0=Alu.mult, op1=Alu.add,
            )
            B[b]["cnt"] = cnt_s
        for b in ("dn", "up"):
            acol = self.small()
            nc.vector.tensor_single_scalar(
                out=acol[:, 0:1], in_=B[b]["cnt"][:, W - 1 : W],
                scalar=float(CH), op=Alu.is_ge,
            )
            par = self.psrow.tile([1, P], F32, tag="psrow")
            self.mm(par[0:1, 0:P], acol[:, 0:1], self.Ident)
            arow = self.row()
            nc.vector.tensor_copy(arow[0:1, 0:P], par[0:1, 0:P])
            self.rels(acol)
            B[b]["arow"] = arow

        def chain(bb, scan1_tile, d0, d1):
            rowt = self.row129()
            nc.gpsimd.memset(rowt[0:1, 0:1], 0.0)
            pbr = self.psrow.tile([1, P], F32, tag="psrow")
            self.mm(pbr[0:1, 0:P], scan1_tile[:, W - 1 : W], self.Ident)
            nc.vector.tensor_tensor_scan(
                out=rowt[0:1, 1 : P + 1], data0=B[bb]["arow"][0:1, 0:P],
                data1=pbr[0:1, 0:P], initial=0.0, op0=Alu.mult, op1=Alu.add,
            )
            pcc = self.pscol.tile([P, 1], F32, tag="pscol")
            self.mm(pcc[:, 0:1], rowt[0:1, 0:P], self.ones11)
            nc.vector.tensor_tensor_scan(
                out=scan1_tile[:, C0:W], data0=d0, data1=d1,
                initial=pcc[:, 0:1], op0=Alu.mult, op1=Alu.add,
            )
            self.relr129(rowt)

        for b in ("dn", "up"):
            chain(b, B[b]["cnt"], B[b]["m"][:, C0:W], B[b]["m"][:, C0:W])
        for b in ("dn", "up"):
            m_ = B[b]["m"]
            Ssum = self.big()
            nc.vector.tensor_tensor_scan(
                out=Ssum[:, C0:W], data0=m_[:, C0:W],
                data1=B[b]["dmask"][:, C0:W],
                initial=0.0, op0=Alu.mult, op1=Alu.add,
            )
            B[b]["S"] = Ssum

        # EMAJX8 lane (ACT+Gp, fills gaps during MA)
        s8jx = self.scan_u(w, 17)
        self.fix_u(s8jx, 17)
        s8f1 = self.scan_u(f1u, 18)
        self.fix_u(s8f1, 18)
        s8f2 = self.scan_u(f2u, 19)
        self.fix_u(s8f2, 19)
        JXbt = self.act_mul(w, 300.0 * a6)
        self.store_row(19, JXbt)
        F1t = self.act_mul(f1u, 300.0 * a8)
        self.store_row(20, F1t)
        F2t = self.act_mul(f2u, 300.0 * a6)
        self.store_row(21, F2t)
        self.rel(w, f1u, f2u, JXbt, F1t, F2t)

        for b in ("dn", "up"):
            chain(b, B[b]["S"], B[b]["m"][:, C0:W], B[b]["dmask"][:, C0:W])
            self.rel(B[b]["m"], B[b]["dmask"])
            self.relr(B[b]["arow"])
        for b in ("dn", "up"):
            seen = self.big()
            dcol = self.small()
            nc.vector.scalar_tensor_tensor(
                out=seen[:, C0:W], in0=B[b]["cnt"][:, C0:W], scalar=1.0,
                in1=self.TG[:, 0:CH], op0=Alu.mult, op1=Alu.is_le,
                accum_out=dcol[:, 0:1],
            )
            B[b]["seen"] = seen
            B[b]["dcol"] = dcol
            rc = self.big()
            nc.vector.tensor_scalar_max(rc[:, C0:W], B[b]["cnt"][:, C0:W], 1.0)
            rcp = self.big()
            nc.vector.reciprocal_approx_fast(out=rcp[:, C0:W], in_=rc[:, C0:W])
            self.rel(rc)
            B[b]["rcp"] = rcp

        # EMAJX8 combine (off path, ACT/Gp)
        e8 = self.act_mul(s8jx, 300.0 * a6 * a17)
        p8 = self.act_mul(s8f2, a6 / a8)
        q8 = self.gp_tt(s8f1, p8, Alu.add)
        self.rel(p8, s8jx, s8f1, s8f2)
        q82 = self.act_affine(q8, 1800.0 * a8 * a18, self.m50[:, 0:1])
        self.rel(q8)
        EMAJX8 = self.gp_tt(e8, q82, Alu.add)
        self.rel(e8, q82)
        self.store_row(29, EMAJX8)
        self.rel(EMAJX8)

        # MA tail (DVE, serial)
        for b, row_idx, diag_idx in (("dn", 25, 1), ("up", 26, 0)):
            ma0 = self.dve_tt(B[b]["S"], B[b]["rcp"], Alu.mult)
            ma = self.dve_tt(ma0, B[b]["seen"], Alu.mult)
            self.rel(ma0, B[b]["cnt"], B[b]["S"], B[b]["rcp"])
            self.store_row(row_idx, ma)
            drow = self.row()
            nc.sync.dma_start(
                out=drow[0:1, 0 : P - 1], in_=B[b]["dcol"][1:P, 0:1]
            )
            done = self.spool.tile([1, 1], F32, tag=f"diag{diag_idx}")
            nc.vector.tensor_reduce(
                out=done[0:1, 0:1], in_=drow[0:1, 0 : P - 1],
                axis=mybir.AxisListType.X, op=Alu.min,
            )
            self.relr(drow)
            nc.sync.dma_start(
                out=self.DIAG[diag_idx : diag_idx + 1].rearrange(
                    "(a b) -> a b", a=1, b=1
                ),
                in_=done[0:1, 0:1],
            )
            self.rels(B[b]["dcol"])
            self.rel(B[b]["seen"], ma)

        self.rel(Ct, JX, EMAJX)


_CACHE = {}


def _build(alphas, anchor):
    key = (tuple(round(float(a), 12) for a in alphas), round(float(anchor), 6))
    if key not in _CACHE:
        kb = KB(alphas, anchor)
        _CACHE[key] = (kb.build(), kb.host_const())
    return _CACHE[key]


def _shard(x):
    outs = []
    for mcore in range(NCORES):
        lo = (mcore + 1) * S - DLEN
        if lo < 0:
            d = np.concatenate(
                [np.full(-lo, x[0], np.float32), x[0 : (mcore + 1) * S]]
            )
        else:
            d = x[lo : (mcore + 1) * S]
        outs.append(np.ascontiguousarray(d, np.float32))
    return outs


def _host_ma(C, JX, EJ):
    f32 = np.float32
    T_ = len(C)
    lag = lambda x: np.concatenate([x[:1], x[:-1]])
    JXp, EJp = lag(JX), lag(EJ)
    res = {}
    cs = np.concatenate([[0.0], np.cumsum(C.astype(np.float64))])
    t_idx = np.arange(T_)
    for key, cond in (
        ("dn", (JX < EJ) & (JXp >= EJp)),
        ("up", (JX > EJ) & (JXp <= EJp)),
    ):
        last = np.maximum.accumulate(np.where(cond, t_idx, -1))
        csl = cs[np.maximum(last, 0) + 1]
        s = cs[t_idx + 1] - csl
        n = t_idx - last
        res[key] = np.where(
            (last >= 0) & (n > 0), s / np.maximum(n, 1), 0.0
        ).astype(f32)
    return res["dn"], res["up"]


def run_cores(inputs, trace=False):
    C = np.ascontiguousarray(inputs["C"], np.float32)
    H = np.ascontiguousarray(inputs["H"], np.float32)
    L = np.ascontiguousarray(inputs["L"], np.float32)
    wv = np.asarray(inputs["w_alphas"], np.float32)
    alphas = [float(1.0 / (1.0 + math.exp(-float(x)))) for x in wv]
    nc, const_arr = _build(alphas, float(C[0]))
    dc, dh, dl = _shard(C), _shard(H), _shard(L)
    in_maps = [
        {"DC": dc[m], "DH": dh[m], "DL": dl[m], "CONST": const_arr}
        for m in range(NCORES)
    ]
    res = run_bass_kernel_spmd(
        nc, in_maps, core_ids=list(range(NCORES)), trace=trace
    )
    return res


def kernel(C, H, L, w_alphas):
    inputs = {"C": C, "H": H, "L": L, "w_alphas": w_alphas}
    res = run_cores(inputs)
    outs = [
        res.results[m]["OUT"].reshape(NROWS, EXT)[:, HALO:] for m in range(NCORES)
    ]
    full = np.concatenate(outs, axis=1)
    full[0] = np.asarray(C, np.float32)
    full[1] = np.asarray(H, np.float32)
    full[2] = np.asarray(L, np.float32)

    Cg = np.asarray(C, np.float64)[:17]
    for t in range(17):
        wdw = Cg[: t + 1]
        dis = math.sqrt(max(np.mean(wdw * wdw) - np.mean(wdw) ** 2, 0.0))
        full[3, t] = np.float32(full[4, t] + dis)
        full[5, t] = np.float32(full[4, t] - dis)

    need_fix = False
    for mcore in range(1, NCORES):
        dg = res.results[mcore]["DIAG"]
        if dg.min() < CH - 0.5:
            need_fix = True
    if need_fix:
        ma_dn, ma_up = _host_ma(np.asarray(C, np.float32), full[27], full[28])
        full[25] = ma_dn
        full[26] = ma_up
    return full.astype(np.float32)
